# revision 38
# baseline (speedup 1.0000x reference)
"""CPC unsupervised criterion loss on 8 Trainium2 NeuronCores.

Strategy (data-parallel over batch B=8, one batch row per core):
  - The irregular 121 MB negative-sample gather is replaced by a dense
    score matrix: for each (k, w) we compute scores against ALL B*S=1024
    encoder rows via PE matmuls in bf16 (4x PE rate). Sampled-negative
    multiplicities cnt[w,j] are built on the host from the index tensors,
    so sum_n exp(negScore_n) = sum_j cnt[w,j]*exp(score[w,j]) and
    max_n negScore_n = max over {j: cnt[w,j]>0} of score[w,j].
  - The PE runs ONLY back-to-back matmuls whose cross-engine inputs were
    produced at least one pipeline round earlier, so it never stalls: a
    stalled PE resets the p-state ramp and pins the clock at 1.2 GHz
    instead of 2.4 GHz (measured: this alone cost the old design ~2x).
  - Engine assignment per k:
      PE:  locC (4 MM) + scores vs all 1024 cols (4 MM) + ONE combined
           bf16 identity-matmul pair adding nbdl = mask(-60000) + ln cnt
           into the score PSUM (start=False accumulate).
      DVE: posS extraction (one-hot dot over the RAW scores, one round
           before the bias matmul lands - no WAR stall), and reduce_max
           over scrB -> maxprod = max over sampled of cnt*e^s; the host
           brackets the true max-neg in [log maxprod - log cmax,
           log maxprod] and re-resolves only in-band pairs exactly.
           (tensor_tensor_reduce would give the exact max in one op but
           dies at NEFF runtime on this stack - do not use it.)
      ACT: Exp over the masked PSUM with accum_out -> negsum comes free
           from the activation accumulator; locC PSUM->bf16 casts
           alternate between ACT and DVE by parity.
  - Software pipeline (4 stages): locC/cast at round r, scores/posS at
    r-1, bias/exp at r-2, maxexp ttr at r-3.
  - Host: loss = log(negsum + e^pos) - pos in f64; acc = pos >=
    log(maxexp), with near-margin pairs (|margin| < tau, includes exact
    ties where the positive was drawn as its own negative) re-resolved
    exactly in f64 via a vectorized dense recompute.
"""

import numpy as np

B, S, K, D, NNEG = 8, 128, 12, 256, 128
W = S - K          # 116
J = B * S          # 1024
NCORES = 8
MASK_NEG = -60000.0      # bf16 ~= -59904, "-inf" for unsampled columns
MARGIN_TAU = 0.15        # host re-check window (covers fp8 score noise)

# fp8 blob columns: flatT (ec-major); bf16 blob: cT (padded to 2x128)
BB_CT = 0
BB_END = 2 * 128                                     # 256
# bf16 mask blob columns: id128 | nbdl(=mask + ln cnt)
MK_ID, MK_NB = 0, 128
MK_END = 128 + J                                     # 1152

_CACHE = {}


def _build_program():
    from concourse import bacc, mybir
    import concourse.tile as tile

    f32 = mybir.dt.float32
    bf16 = mybir.dt.bfloat16
    fp8 = mybir.dt.float8e4
    Alu = mybir.AluOpType
    Act = mybir.ActivationFunctionType

    nc = bacc.Bacc(
        "TRN2", target_bir_lowering=False, debug=False, num_devices=NCORES
    )

    fb_d = nc.dram_tensor("fblob", [128, S + K], f32, kind="ExternalInput")
    bb_d = nc.dram_tensor("bblob", [128, BB_END], bf16, kind="ExternalInput")
    f8_d = nc.dram_tensor("f8blob", [128, 2 * J], fp8, kind="ExternalInput")
    mk_d = nc.dram_tensor("mblob", [128, MK_END], bf16, kind="ExternalInput")
    wp_d = nc.dram_tensor("wpredT", [128, K * 2 * D], fp8, kind="ExternalInput")
    out_d = nc.dram_tensor("out", [128, 3 * K], f32, kind="ExternalOutput")

    with tile.TileContext(nc) as tc:
        with (
            tc.tile_pool(name="consts", bufs=1) as consts,
            tc.tile_pool(name="lcpool", bufs=3) as lcpool,
            tc.tile_pool(name="scr", bufs=3) as scr,
            tc.tile_pool(name="junk", bufs=2) as junkp,
            tc.tile_pool(name="outs", bufs=1) as outs,
            tc.tile_pool(name="pslc", bufs=2, space="PSUM") as pslc,
            tc.tile_pool(name="pssc", bufs=3, space="PSUM") as pssc,
        ):
            bb = consts.tile([128, BB_END], bf16)
            f8b = consts.tile([128, 2 * J], fp8)
            wpall = consts.tile([128, K * 2 * D], fp8)
            fb = consts.tile([128, S + K], f32)
            mk = consts.tile([128, MK_END], bf16)
            # DMA order matches round-k consumption (cT+wp first, then fT
            # for scores 0, mk for bias 0, fb for posS 0). Each dma_start
            # costs ~600ns of serial issue time on its engine, so issues
            # are spread across engines to queue them in parallel.
            nc.sync.dma_start(bb[:], bb_d[:])
            nc.sync.dma_start(wpall[:, 0:1024], wp_d[:, 0:1024])
            nc.scalar.dma_start(f8b[:, 0:J], f8_d[:, 0:J])
            nc.scalar.dma_start(f8b[:, J:2 * J], f8_d[:, J:2 * J])
            nc.sync.dma_start(wpall[:, 1024:3072], wp_d[:, 1024:3072])
            nc.scalar.dma_start(mk[:], mk_d[:])
            nc.scalar.dma_start(fb[:], fb_d[:])
            nc.sync.dma_start(wpall[:, 3072:K * 512], wp_d[:, 3072:K * 512])

            fT_v = f8b[:, 0:2 * J]
            cT_v = bb[:, BB_CT:BB_CT + 2 * 128]
            id_v = mk[:, MK_ID:MK_ID + 128]
            nb_v = mk[:, MK_NB:MK_NB + J]

            outt = outs.tile([128, 3 * K], f32)
            negsum = outt[:, 0:K]
            posS = outt[:, K:2 * K]
            maxexp = outt[:, 2 * K:3 * K]

            lc_ps_t, lc_bf_t, sc_t, scrB_t = {}, {}, {}, {}

            def emit_locC(k):
                t = pslc.tile([128, 256], f32, tag="lcT")
                lc_ps_t[k] = t
                wk = wpall[:, k * 2 * D:(k + 1) * 2 * D]
                for ec in range(2):
                    for dc in range(2):
                        nc.tensor.matmul(
                            t[:, ec * 128:(ec + 1) * 128],
                            lhsT=wk[:, dc * D + ec * 128: dc * D + (ec + 1) * 128],
                            rhs=cT_v[:, dc * 128:(dc + 1) * 128],
                            start=(dc == 0),
                            stop=(dc == 1),
                        )

            def emit_cast(k):
                t = lcpool.tile([128, 256], fp8, tag="lcT_f8")
                lc_bf_t[k] = t
                if k % 2 == 0:
                    nc.scalar.copy(t[:], lc_ps_t[k][:])
                else:
                    nc.vector.tensor_copy(t[:], lc_ps_t[k][:])
                del lc_ps_t[k]

            def emit_scores(k):
                t = pssc.tile([128, J], f32, tag="sc")
                sc_t[k] = t
                lcb = lc_bf_t[k]
                for jc in range(2):
                    for ec in range(2):
                        nc.tensor.matmul(
                            t[:, jc * 512:(jc + 1) * 512],
                            lhsT=lcb[:, ec * 128:(ec + 1) * 128],
                            rhs=fT_v[:, ec * J + jc * 512: ec * J + (jc + 1) * 512],
                            start=(ec == 0),
                            stop=False,
                            skip_group_check=True,
                        )

            def emit_posS(k):
                scrP = junkp.tile([128, S], f32, tag="scrP")
                nc.vector.scalar_tensor_tensor(
                    out=scrP[:],
                    in0=sc_t[k][:, 0:S],
                    scalar=1.0,
                    in1=fb[:, K - k:K - k + S],
                    op0=Alu.mult,
                    op1=Alu.mult,
                    accum_out=posS[:, k:k + 1],
                )

            def emit_bias(k):
                for jc in range(2):
                    nc.tensor.matmul(
                        sc_t[k][:, jc * 512:(jc + 1) * 512],
                        lhsT=id_v,
                        rhs=nb_v[:, jc * 512:(jc + 1) * 512],
                        start=False,
                        stop=(jc == 1),
                        skip_group_check=True,
                    )

            def emit_exp(k):
                scrB = scr.tile([128, J], bf16, tag="scrB")
                scrB_t[k] = scrB
                nc.scalar.activation(
                    out=scrB[:],
                    in_=sc_t[k][:],
                    func=Act.Exp,
                    accum_out=negsum[:, k:k + 1],
                )
                del sc_t[k]

            def emit_maxexp(k):
                nc.vector.reduce_max(
                    maxexp[:, k:k + 1], scrB_t[k][:], axis=mybir.AxisListType.X
                )
                del scrB_t[k]

            # Warmup: dummy matmuls on cT (first DMA to land) keep the PE
            # continuously busy through the DMA lead-in, so the 3us
            # p-state ramp to 2.4 GHz completes before the real rounds
            # (a cold or stalled PE streams at 1.2 GHz). Tiles come from
            # the locC PSUM ring; WAW ordering is free since the PE is
            # in-order anyway.
            for _ in range(8):
                warm = pslc.tile([128, 256], f32, tag="lcT")
                nc.tensor.matmul(
                    warm[:],
                    lhsT=cT_v[:, 0:128],
                    rhs=cT_v[:, 0:256],
                    start=True,
                    stop=True,
                )

            # software pipeline, 4 stages:
            #   r: locC/cast | r-1: scores/posS | r-2: bias/exp | r-3: ttr
            for r in range(K + 3):
                if r < K:
                    emit_locC(r)
                    emit_cast(r)
                if 1 <= r <= K:
                    k = r - 1
                    emit_scores(k)
                    emit_posS(k)
                if 2 <= r <= K + 1:
                    k = r - 2
                    emit_bias(k)
                    emit_exp(k)
                if r >= 3:
                    emit_maxexp(r - 3)

            nc.sync.dma_start(out_d[:], outt[:])

    nc.compile()
    return nc


def _host_prep(cFeature, encodedData, Wpred, batchIdx, seqIdx):
    import ml_dtypes

    bf = ml_dtypes.bfloat16
    f8 = ml_dtypes.float8_e4m3
    cF = np.ascontiguousarray(np.asarray(cFeature, dtype=np.float32))
    eD = np.ascontiguousarray(np.asarray(encodedData, dtype=np.float32))
    Wp = np.ascontiguousarray(np.asarray(Wpred, dtype=np.float32))
    bI = np.asarray(batchIdx).astype(np.int64)
    sI = np.asarray(seqIdx).astype(np.int64)

    flat = eD.reshape(J, D)
    idx = np.arange(NNEG * W * B, dtype=np.int64)
    ext = ((sI + idx % W) % S + bI * S).reshape(B, NNEG, W)

    wt = Wp.transpose(0, 2, 1)  # (K, d, e)
    wp_host = np.concatenate(
        [np.concatenate([wt[k, :128, :], wt[k, 128:, :]], axis=1) for k in range(K)],
        axis=1,
    ).astype(f8)  # (128, K*2D)
    wp_host = np.ascontiguousarray(wp_host)

    # one-hot for extracting pos_w = sc[w, k+1+w]: row w has a 1 at
    # column (k+1+w); view fb[:, K-k:K-k+S] selects it per k
    fblob = np.zeros((128, S + K), np.float32)
    fblob[np.arange(W), np.arange(W) + K + 1] = 1.0

    rows = np.tile(np.arange(W), NNEG)
    in_maps = []
    cnts_orig = []
    for b in range(B):
        perm = np.r_[b * S:(b + 1) * S, 0:b * S, (b + 1) * S:J]
        inv = np.empty(J, np.int64)
        inv[perm] = np.arange(J)

        fT = flat[perm].T  # (D, J) fp32
        cT = cF[b, :W].T * np.float32(1.0 / 256.0)  # exact power-of-2 scale

        cnt = np.zeros((W, J), np.float32)
        np.add.at(cnt, (rows, inv[ext[b].ravel()]), 1.0)
        cnt_o = np.zeros((W, J), np.float32)
        np.add.at(cnt_o, (rows, ext[b].ravel()), 1.0)
        cnts_orig.append(cnt_o)
        nz = cnt > 0

        bblob = np.zeros((128, BB_END), bf)
        bblob[:, BB_CT:BB_CT + W] = cT[:128].astype(bf)
        bblob[:, BB_CT + 128:BB_CT + 128 + W] = cT[128:].astype(bf)
        f8blob = np.zeros((128, 2 * J), f8)
        f8blob[:, 0:J] = fT[:128].astype(f8)
        f8blob[:, J:2 * J] = fT[128:].astype(f8)

        mblob = np.zeros((128, MK_END), bf)
        mblob[:, MK_ID:MK_ID + 128] = np.eye(128, dtype=np.float32).astype(bf)
        nbdl = np.full((W, J), MASK_NEG, np.float32)
        nbdl[nz] = np.log(cnt[nz])
        mblob[:W, MK_NB:MK_NB + J] = nbdl.astype(bf)

        in_maps.append({
            "fblob": fblob,
            "bblob": np.ascontiguousarray(bblob),
            "f8blob": np.ascontiguousarray(f8blob),
            "mblob": np.ascontiguousarray(mblob),
            "wpredT": wp_host,
        })
    return in_maps, cnts_orig, flat, cF, Wp


def _host_fix_acc(acc01, flags, cnts_orig, flat, cF, Wp):
    """Re-resolve flagged accuracy bits exactly in float64 (vectorized)."""
    flagged = np.nonzero(flags)  # (b, w, k) triples
    if flagged[0].size == 0:
        return acc01
    fb_, fw_, fk_ = flagged
    flat64 = flat.astype(np.float64)
    c64 = cF.astype(np.float64) / 256.0
    Wp64 = Wp.astype(np.float64)
    for k in range(K):
        sel = fk_ == k
        if not sel.any():
            continue
        bs, ws = fb_[sel], fw_[sel]
        lc = c64[bs, ws] @ Wp64[k].T                    # (n, 256)
        sc = lc @ flat64.T                              # (n, 1024)
        msk = np.stack([cnts_orig[b][w] > 0 for b, w in zip(bs, ws)])
        mn = np.where(msk, sc, -np.inf).max(axis=1)
        p = sc[np.arange(len(bs)), bs * S + k + 1 + ws]
        acc01[bs, ws, k] = (p >= mn).astype(np.float32)
    return acc01


def kernel(cFeature, encodedData, Wpred, batchIdx, seqIdx, _trace=False):
    from concourse.bass_utils import run_bass_kernel_spmd

    in_maps, cnts_orig, flat, cF, Wp = _host_prep(
        cFeature, encodedData, Wpred, batchIdx, seqIdx
    )

    if "nc" not in _CACHE:
        _CACHE["nc"] = _build_program()
    nc = _CACHE["nc"]

    kw = {}
    if _trace:
        kw = {"trace": True}
    res = run_bass_kernel_spmd(nc, in_maps, core_ids=list(range(NCORES)), **kw)
    _CACHE["last_results"] = res

    outs = np.stack([res.results[b]["out"][:W] for b in range(B)])  # (B, W, 3K)
    negsum = outs[:, :, :K].astype(np.float64)
    posS = outs[:, :, K:2 * K]
    maxprod = outs[:, :, 2 * K:3 * K].astype(np.float64)
    p64 = posS.astype(np.float64)
    lossc = np.log(negsum + np.exp(p64)) - p64

    # maxprod = max over sampled of cnt_j * e^{s_j}, so the true max-neg
    # score lies in [log(maxprod) - log(cmax), log(maxprod)]; decide acc
    # outside that band, re-resolve exactly inside it
    up = np.log(maxprod)                                  # (B, W, K)
    lncmax = np.log(np.stack([c.max(axis=1) for c in cnts_orig]))  # (B, W)
    lo = up - lncmax[:, :, None]
    acc01 = (p64 >= up).astype(np.float32)
    flags = (p64 >= lo - MARGIN_TAU) & (p64 < up + MARGIN_TAU)
    acc01 = _host_fix_acc(acc01, flags, cnts_orig, flat, cF, Wp)

    losses = lossc.sum(axis=(0, 1), dtype=np.float64) / (B * W)
    accs = acc01.sum(axis=(0, 1), dtype=np.float64) / (B * W)
    return (
        losses.astype(np.float32)[None, :],
        accs.astype(np.float32)[None, :],
    )


# revision 39
# speedup vs baseline: 1.0515x; 1.0515x over previous
"""CPC unsupervised criterion loss on 8 Trainium2 NeuronCores.

Strategy (data-parallel over batch B=8, one batch row per core):
  - The irregular 121 MB negative-sample gather is replaced by a dense
    score matrix: for each (k, w) we compute scores against ALL B*S=1024
    encoder rows via PE matmuls in bf16 (4x PE rate). Sampled-negative
    multiplicities cnt[w,j] are built on the host from the index tensors,
    so sum_n exp(negScore_n) = sum_j cnt[w,j]*exp(score[w,j]) and
    max_n negScore_n = max over {j: cnt[w,j]>0} of score[w,j].
  - The PE runs ONLY back-to-back matmuls whose cross-engine inputs were
    produced at least one pipeline round earlier, so it never stalls: a
    stalled PE resets the p-state ramp and pins the clock at 1.2 GHz
    instead of 2.4 GHz (measured: this alone cost the old design ~2x).
  - Engine assignment per k:
      PE:  locC (4 MM) + scores vs all 1024 cols (4 MM) + ONE combined
           bf16 identity-matmul pair adding nbdl = mask(-60000) + ln cnt
           into the score PSUM (start=False accumulate).
      DVE: posS extraction (one-hot dot over the RAW scores, one round
           before the bias matmul lands - no WAR stall), and reduce_max
           over scrB -> maxprod = max over sampled of cnt*e^s; the host
           brackets the true max-neg in [log maxprod - log cmax,
           log maxprod] and re-resolves only in-band pairs exactly.
           (tensor_tensor_reduce would give the exact max in one op but
           dies at NEFF runtime on this stack - do not use it.)
      ACT: Exp over the masked PSUM with accum_out -> negsum comes free
           from the activation accumulator; locC PSUM->bf16 casts
           alternate between ACT and DVE by parity.
  - Software pipeline (4 stages): locC/cast at round r, scores/posS at
    r-1, bias/exp at r-2, maxexp ttr at r-3.
  - Host: loss = log(negsum + e^pos) - pos in f64; acc = pos >=
    log(maxexp), with near-margin pairs (|margin| < tau, includes exact
    ties where the positive was drawn as its own negative) re-resolved
    exactly in f64 via a vectorized dense recompute.
"""

import numpy as np

B, S, K, D, NNEG = 8, 128, 12, 256, 128
W = S - K          # 116
J = B * S          # 1024
NCORES = 8
MASK_NEG = -60000.0      # bf16 ~= -59904, "-inf" for unsampled columns
MARGIN_TAU = 0.15        # host re-check window (covers fp8 score noise)

# fp8 blob columns: flatT (ec-major); bf16 blob: cT (padded to 2x128)
BB_CT = 0
BB_END = 2 * 128                                     # 256
# bf16 mask blob columns: id128 | nbdl(=mask + ln cnt)
MK_ID, MK_NB = 0, 128
MK_END = 128 + J                                     # 1152

_CACHE = {}


def _build_program():
    from concourse import bacc, mybir
    import concourse.tile as tile

    f32 = mybir.dt.float32
    bf16 = mybir.dt.bfloat16
    fp8 = mybir.dt.float8e4
    Alu = mybir.AluOpType
    Act = mybir.ActivationFunctionType

    nc = bacc.Bacc(
        "TRN2", target_bir_lowering=False, debug=False, num_devices=NCORES
    )

    fb_d = nc.dram_tensor("fblob", [128, S + K], f32, kind="ExternalInput")
    bb_d = nc.dram_tensor("bblob", [128, BB_END], bf16, kind="ExternalInput")
    f8_d = nc.dram_tensor("f8blob", [128, 2 * J], fp8, kind="ExternalInput")
    mk_d = nc.dram_tensor("mblob", [128, MK_END], bf16, kind="ExternalInput")
    wp_d = nc.dram_tensor("wpredT", [128, K * 2 * D], fp8, kind="ExternalInput")
    out_d = nc.dram_tensor("out", [128, 3 * K], f32, kind="ExternalOutput")

    with tile.TileContext(nc) as tc:
        with (
            tc.tile_pool(name="consts", bufs=1) as consts,
            tc.tile_pool(name="lcpool", bufs=3) as lcpool,
            tc.tile_pool(name="scr", bufs=3) as scr,
            tc.tile_pool(name="junk", bufs=2) as junkp,
            tc.tile_pool(name="outs", bufs=1) as outs,
            tc.tile_pool(name="pslc", bufs=2, space="PSUM") as pslc,
            tc.tile_pool(name="pssc", bufs=3, space="PSUM") as pssc,
        ):
            bb = consts.tile([128, BB_END], bf16)
            f8b = consts.tile([128, 2 * J], fp8)
            wpall = consts.tile([128, K * 2 * D], fp8)
            fb = consts.tile([128, S + K], f32)
            mk = consts.tile([128, MK_END], bf16)
            # DMA order matches round-k consumption (cT+wp first, then fT
            # for scores 0, mk for bias 0, fb for posS 0). Each dma_start
            # costs ~600ns of serial issue time on its engine, so issues
            # are spread across engines to queue them in parallel.
            nc.sync.dma_start(bb[:], bb_d[:])
            nc.sync.dma_start(wpall[:, 0:1024], wp_d[:, 0:1024])
            nc.scalar.dma_start(f8b[:, 0:J], f8_d[:, 0:J])
            nc.scalar.dma_start(f8b[:, J:2 * J], f8_d[:, J:2 * J])
            nc.sync.dma_start(wpall[:, 1024:3072], wp_d[:, 1024:3072])
            nc.gpsimd.dma_start(mk[:], mk_d[:])
            nc.gpsimd.dma_start(fb[:], fb_d[:])
            nc.sync.dma_start(wpall[:, 3072:K * 512], wp_d[:, 3072:K * 512])

            fT_v = f8b[:, 0:2 * J]
            cT_v = bb[:, BB_CT:BB_CT + 2 * 128]
            id_v = mk[:, MK_ID:MK_ID + 128]
            nb_v = mk[:, MK_NB:MK_NB + J]

            outt = outs.tile([128, 3 * K], f32)
            negsum = outt[:, 0:K]
            posS = outt[:, K:2 * K]
            maxexp = outt[:, 2 * K:3 * K]

            lc_ps_t, lc_bf_t, sc_t, scrB_t = {}, {}, {}, {}

            def emit_locC(k):
                t = pslc.tile([128, 256], f32, tag="lcT")
                lc_ps_t[k] = t
                wk = wpall[:, k * 2 * D:(k + 1) * 2 * D]
                for ec in range(2):
                    for dc in range(2):
                        nc.tensor.matmul(
                            t[:, ec * 128:(ec + 1) * 128],
                            lhsT=wk[:, dc * D + ec * 128: dc * D + (ec + 1) * 128],
                            rhs=cT_v[:, dc * 128:(dc + 1) * 128],
                            start=(dc == 0),
                            stop=(dc == 1),
                        )

            def emit_cast(k):
                t = lcpool.tile([128, 256], fp8, tag="lcT_f8")
                lc_bf_t[k] = t
                if k % 2 == 0:
                    nc.scalar.copy(t[:], lc_ps_t[k][:])
                else:
                    nc.vector.tensor_copy(t[:], lc_ps_t[k][:])
                del lc_ps_t[k]

            def emit_scores(k):
                t = pssc.tile([128, J], f32, tag="sc")
                sc_t[k] = t
                lcb = lc_bf_t[k]
                for jc in range(2):
                    for ec in range(2):
                        nc.tensor.matmul(
                            t[:, jc * 512:(jc + 1) * 512],
                            lhsT=lcb[:, ec * 128:(ec + 1) * 128],
                            rhs=fT_v[:, ec * J + jc * 512: ec * J + (jc + 1) * 512],
                            start=(ec == 0),
                            stop=False,
                            skip_group_check=True,
                        )

            def emit_posS(k):
                scrP = junkp.tile([128, S], f32, tag="scrP")
                nc.vector.scalar_tensor_tensor(
                    out=scrP[:],
                    in0=sc_t[k][:, 0:S],
                    scalar=1.0,
                    in1=fb[:, K - k:K - k + S],
                    op0=Alu.mult,
                    op1=Alu.mult,
                    accum_out=posS[:, k:k + 1],
                )

            def emit_bias(k):
                for jc in range(2):
                    nc.tensor.matmul(
                        sc_t[k][:, jc * 512:(jc + 1) * 512],
                        lhsT=id_v,
                        rhs=nb_v[:, jc * 512:(jc + 1) * 512],
                        start=False,
                        stop=(jc == 1),
                        skip_group_check=True,
                    )

            def emit_exp(k):
                scrB = scr.tile([128, J], bf16, tag="scrB")
                scrB_t[k] = scrB
                nc.scalar.activation(
                    out=scrB[:],
                    in_=sc_t[k][:],
                    func=Act.Exp,
                    accum_out=negsum[:, k:k + 1],
                )
                del sc_t[k]

            def emit_maxexp(k):
                nc.vector.reduce_max(
                    maxexp[:, k:k + 1], scrB_t[k][:], axis=mybir.AxisListType.X
                )
                del scrB_t[k]

            # Warmup: dummy matmuls on cT (first DMA to land) keep the PE
            # continuously busy through the DMA lead-in, so the 3us
            # p-state ramp to 2.4 GHz completes before the real rounds
            # (a cold or stalled PE streams at 1.2 GHz). Tiles come from
            # the locC PSUM ring; WAW ordering is free since the PE is
            # in-order anyway.
            for _ in range(8):
                warm = pslc.tile([128, 256], f32, tag="lcT")
                nc.tensor.matmul(
                    warm[:],
                    lhsT=cT_v[:, 0:128],
                    rhs=cT_v[:, 0:256],
                    start=True,
                    stop=True,
                )

            # software pipeline, 4 stages:
            #   r: locC/cast | r-1: scores/posS | r-2: bias/exp | r-3: ttr
            for r in range(K + 3):
                if r < K:
                    emit_locC(r)
                    emit_cast(r)
                if 1 <= r <= K:
                    k = r - 1
                    emit_scores(k)
                    emit_posS(k)
                if 2 <= r <= K + 1:
                    k = r - 2
                    emit_bias(k)
                    emit_exp(k)
                if r >= 3:
                    emit_maxexp(r - 3)

            nc.sync.dma_start(out_d[:], outt[:])

    nc.compile()
    return nc


def _host_prep(cFeature, encodedData, Wpred, batchIdx, seqIdx):
    import ml_dtypes

    bf = ml_dtypes.bfloat16
    f8 = ml_dtypes.float8_e4m3
    cF = np.ascontiguousarray(np.asarray(cFeature, dtype=np.float32))
    eD = np.ascontiguousarray(np.asarray(encodedData, dtype=np.float32))
    Wp = np.ascontiguousarray(np.asarray(Wpred, dtype=np.float32))
    bI = np.asarray(batchIdx).astype(np.int64)
    sI = np.asarray(seqIdx).astype(np.int64)

    flat = eD.reshape(J, D)
    idx = np.arange(NNEG * W * B, dtype=np.int64)
    ext = ((sI + idx % W) % S + bI * S).reshape(B, NNEG, W)

    wt = Wp.transpose(0, 2, 1)  # (K, d, e)
    wp_host = np.concatenate(
        [np.concatenate([wt[k, :128, :], wt[k, 128:, :]], axis=1) for k in range(K)],
        axis=1,
    ).astype(f8)  # (128, K*2D)
    wp_host = np.ascontiguousarray(wp_host)

    # one-hot for extracting pos_w = sc[w, k+1+w]: row w has a 1 at
    # column (k+1+w); view fb[:, K-k:K-k+S] selects it per k
    fblob = np.zeros((128, S + K), np.float32)
    fblob[np.arange(W), np.arange(W) + K + 1] = 1.0

    rows = np.tile(np.arange(W), NNEG)
    in_maps = []
    cnts_orig = []
    for b in range(B):
        perm = np.r_[b * S:(b + 1) * S, 0:b * S, (b + 1) * S:J]
        inv = np.empty(J, np.int64)
        inv[perm] = np.arange(J)

        fT = flat[perm].T  # (D, J) fp32
        cT = cF[b, :W].T * np.float32(1.0 / 256.0)  # exact power-of-2 scale

        cnt = np.zeros((W, J), np.float32)
        np.add.at(cnt, (rows, inv[ext[b].ravel()]), 1.0)
        cnt_o = np.zeros((W, J), np.float32)
        np.add.at(cnt_o, (rows, ext[b].ravel()), 1.0)
        cnts_orig.append(cnt_o)
        nz = cnt > 0

        bblob = np.zeros((128, BB_END), bf)
        bblob[:, BB_CT:BB_CT + W] = cT[:128].astype(bf)
        bblob[:, BB_CT + 128:BB_CT + 128 + W] = cT[128:].astype(bf)
        f8blob = np.zeros((128, 2 * J), f8)
        f8blob[:, 0:J] = fT[:128].astype(f8)
        f8blob[:, J:2 * J] = fT[128:].astype(f8)

        mblob = np.zeros((128, MK_END), bf)
        mblob[:, MK_ID:MK_ID + 128] = np.eye(128, dtype=np.float32).astype(bf)
        nbdl = np.full((W, J), MASK_NEG, np.float32)
        nbdl[nz] = np.log(cnt[nz])
        mblob[:W, MK_NB:MK_NB + J] = nbdl.astype(bf)

        in_maps.append({
            "fblob": fblob,
            "bblob": np.ascontiguousarray(bblob),
            "f8blob": np.ascontiguousarray(f8blob),
            "mblob": np.ascontiguousarray(mblob),
            "wpredT": wp_host,
        })
    return in_maps, cnts_orig, flat, cF, Wp


def _host_fix_acc(acc01, flags, cnts_orig, flat, cF, Wp):
    """Re-resolve flagged accuracy bits exactly in float64 (vectorized)."""
    flagged = np.nonzero(flags)  # (b, w, k) triples
    if flagged[0].size == 0:
        return acc01
    fb_, fw_, fk_ = flagged
    flat64 = flat.astype(np.float64)
    c64 = cF.astype(np.float64) / 256.0
    Wp64 = Wp.astype(np.float64)
    for k in range(K):
        sel = fk_ == k
        if not sel.any():
            continue
        bs, ws = fb_[sel], fw_[sel]
        lc = c64[bs, ws] @ Wp64[k].T                    # (n, 256)
        sc = lc @ flat64.T                              # (n, 1024)
        msk = np.stack([cnts_orig[b][w] > 0 for b, w in zip(bs, ws)])
        mn = np.where(msk, sc, -np.inf).max(axis=1)
        p = sc[np.arange(len(bs)), bs * S + k + 1 + ws]
        acc01[bs, ws, k] = (p >= mn).astype(np.float32)
    return acc01


def kernel(cFeature, encodedData, Wpred, batchIdx, seqIdx, _trace=False):
    from concourse.bass_utils import run_bass_kernel_spmd

    in_maps, cnts_orig, flat, cF, Wp = _host_prep(
        cFeature, encodedData, Wpred, batchIdx, seqIdx
    )

    if "nc" not in _CACHE:
        _CACHE["nc"] = _build_program()
    nc = _CACHE["nc"]

    kw = {}
    if _trace:
        kw = {"trace": True}
    res = run_bass_kernel_spmd(nc, in_maps, core_ids=list(range(NCORES)), **kw)
    _CACHE["last_results"] = res

    outs = np.stack([res.results[b]["out"][:W] for b in range(B)])  # (B, W, 3K)
    negsum = outs[:, :, :K].astype(np.float64)
    posS = outs[:, :, K:2 * K]
    maxprod = outs[:, :, 2 * K:3 * K].astype(np.float64)
    p64 = posS.astype(np.float64)
    lossc = np.log(negsum + np.exp(p64)) - p64

    # maxprod = max over sampled of cnt_j * e^{s_j}, so the true max-neg
    # score lies in [log(maxprod) - log(cmax), log(maxprod)]; decide acc
    # outside that band, re-resolve exactly inside it
    up = np.log(maxprod)                                  # (B, W, K)
    lncmax = np.log(np.stack([c.max(axis=1) for c in cnts_orig]))  # (B, W)
    lo = up - lncmax[:, :, None]
    acc01 = (p64 >= up).astype(np.float32)
    flags = (p64 >= lo - MARGIN_TAU) & (p64 < up + MARGIN_TAU)
    acc01 = _host_fix_acc(acc01, flags, cnts_orig, flat, cF, Wp)

    losses = lossc.sum(axis=(0, 1), dtype=np.float64) / (B * W)
    accs = acc01.sum(axis=(0, 1), dtype=np.float64) / (B * W)
    return (
        losses.astype(np.float32)[None, :],
        accs.astype(np.float32)[None, :],
    )
